# revision 17
# baseline (speedup 1.0000x reference)
"""Trainium2 Bass kernel for nn_BiMP (GNN message passing), 8 NeuronCores SPMD.

v2 rewrite. Per-core pipeline:
  P1-A: kv = x_c @ [k1|v1] + b  (bf16 matmuls, node-major)  -> kv_loc -> AllGather
  P1-B: qq = x_c @ [q1*isq | qWe | skip1] + b -> q_loc (rows [q|qWe|0..]) + skip in SBUF
  edge phase (dst-sorted edges, window = 128 dst nodes, padded to cap=ntile*128):
    dma_gather kv rows by src (512B rows), q rows by dst from q_loc;
    alpha = sum_c q*k per head + ea*qWe  (DVE, half-window batches);
    ex = exp(alpha) bf16 (no max-subtract; alpha is O(+-12));
    rhs = [ex | ex*ea | v*ex]; agg += S_j.T @ rhs_j (one-hot scatter matmuls);
    h = (aggv + (sum ex*ea)*We)/denom + skip; transpose -> hT bf16 -> AllGather
  gn1 stats redundant per core on gathered hT; relu-normalize (ACT, feature-major)
  stage 2 (dense bipartite attention, feature-major):
    k2T = k2w.T @ hrelu; q2T/skipT from target embeddings; per s-tile:
    v2a aug [v_h|1], scoresT = k2T_h.T @ q2T_h (tile_position row groups),
    exp (ACT), nd_h += v2a_h.T @ exp_h (numerator+denominator);
    xtpT = nd/denom + skipT; gn2 stats AllReduce; xtnT = relu-normalize;
  AllGather xtnT -> adj = xtnT.T @ xtfT (K=128 matmuls); minmax AllReduce; normalize.
Self-contained: hardcodes shapes; compiles on first call (cached per ntile).
"""
import os
import sys
import types

import numpy as np


def _install_ntff_shim():
    """bass_utils imports antenv.axon_hooks when tracing; provide it."""
    if "antenv.axon_hooks" in sys.modules:
        return
    mod = types.ModuleType("antenv.axon_hooks")

    def set_axon_ntff_profile_hook(h):
        mod._hook = h

    def get_axon_ntff_profile_hook():
        return getattr(mod, "_hook", None)

    mod.set_axon_ntff_profile_hook = set_axon_ntff_profile_hook
    mod.get_axon_ntff_profile_hook = get_axon_ntff_profile_hook
    sys.modules["antenv.axon_hooks"] = mod
    try:
        import antenv
        antenv.axon_hooks = mod
        from trn_agent_boot.trn_boot import _ntff_profile_via_ctypes
        set_axon_ntff_profile_hook(_ntff_profile_via_ctypes("/opt/axon/libaxon_pjrt.so"))
    except Exception:
        pass


_install_ntff_shim()

import ml_dtypes
import concourse.bacc as bacc
import concourse.bass as bass
import concourse.mybir as mybir
import concourse.tile as tile
from concourse.bass_utils import run_bass_kernel_spmd
from concourse.masks import make_identity

dt = mybir.dt
bf16 = ml_dtypes.bfloat16
AF = mybir.ActivationFunctionType
OP = mybir.AluOpType
AX = mybir.AxisListType

NS, NT, H, C = 4096, 2048, 4, 32
D = H * C            # 128
E1 = 131072
M = 8                # cores
NSL = NS // M        # 512 source nodes / core
NTL = NT // M        # 256 target rows / core
WIN = 128            # dst nodes per window
NWIN = NSL // WIN    # 4 windows / core
P = 128
ISQ = np.float32(1.0 / np.sqrt(np.float32(C)))
EPS_GN = np.float32(1e-5)

_prog_cache = {}


# --------------------------------------------------------------------------
# host-side preparation
# --------------------------------------------------------------------------

def _wrap_idx(idx):
    """int16 idx array [n] -> [128, n//16] wrapped layout for dma_gather
    (idx i read from partition 16+(i%16), col i//16; replicate all groups)."""
    n = idx.shape[0]
    blk = idx.reshape(n // 16, 16).T.astype(np.int16)   # [16, n//16]
    return np.tile(blk, (8, 1))                          # [128, n//16]


def _prep(inputs):
    f32 = lambda k: np.asarray(inputs[k], np.float32)
    x = f32("x")
    src = np.asarray(inputs["pos_edge_index"][0]).astype(np.int64)
    dst = np.asarray(inputs["pos_edge_index"][1]).astype(np.int64)
    ea = f32("edge_attr").reshape(-1)
    xt_emb = f32("target_node_embeddings")

    We = f32("e1_w").reshape(D)
    M2T = np.zeros((D, H), np.float32)
    for h in range(H):
        M2T[h * C:(h + 1) * C, h] = We[h * C:(h + 1) * C]
    Wq_s = f32("q1_w") * ISQ
    W1 = np.ascontiguousarray(
        np.concatenate([Wq_s, Wq_s @ M2T, f32("skip1_w")], axis=1)).astype(bf16)
    W2 = np.ascontiguousarray(
        np.concatenate([f32("k1_w"), f32("v1_w")], axis=1)).astype(bf16)
    bq = f32("q1_b") * ISQ
    B1 = np.concatenate([bq, bq @ M2T, f32("skip1_b")]).reshape(1, 260).astype(bf16)
    B2 = np.concatenate([f32("k1_b"), f32("v1_b")]).reshape(1, 256).astype(bf16)

    order = np.argsort(dst, kind="stable")
    src_s, dst_s, ea_s = src[order], dst[order], ea[order]
    win_id = dst_s // WIN
    counts = np.bincount(win_id, minlength=NS // WIN)
    cap = int(np.ceil(max(int(counts.max()), 128) / 128) * 128)
    ntile = cap // 128
    starts = np.zeros(NS // WIN + 1, np.int64)
    np.cumsum(counts, out=starts[1:])

    xT = x.T
    shared = {
        "W1": W1, "W2": W2, "B1": B1, "B2": B2,
        "We_row": We.reshape(1, D).astype(np.float32),
        "q2w": np.ascontiguousarray(f32("q2_w") * ISQ).astype(bf16),
        "q2b": (f32("q2_b") * ISQ).reshape(1, D).astype(bf16),
        "k2w": f32("k2_w").astype(bf16),
        "k2b": f32("k2_b").reshape(1, D).astype(bf16),
        "sk2w": f32("skip2_w").astype(bf16),
        "sk2b": f32("skip2_b").reshape(1, D).astype(bf16),
        "gn1_cols": np.stack([f32("gn1_w"), f32("gn1_b"), f32("gn1_ms")], axis=1),
        "gn2_cols": np.stack([f32("gn2_w"), f32("gn2_b"), f32("gn2_ms")], axis=1),
        "ones_bf": np.ones((1, 512), bf16),
    }
    # v2 aug: [v2_h | 1-slot] x 4 heads -> [128, 132]
    v2wa = np.zeros((D, 4 * 33), np.float32)
    v2ba = np.zeros((1, 4 * 33), np.float32)
    v2w_np, v2b_np = f32("v2_w"), f32("v2_b")
    for h in range(H):
        v2wa[:, 33 * h:33 * h + 32] = v2w_np[:, 32 * h:32 * (h + 1)]
        v2ba[0, 33 * h:33 * h + 32] = v2b_np[32 * h:32 * (h + 1)]
        v2ba[0, 33 * h + 32] = 1.0
    shared["v2wa"] = v2wa.astype(bf16)
    shared["v2ba"] = v2ba.astype(bf16)

    per_core = []
    for c in range(M):
        kidx = np.zeros((NWIN, cap), np.int64)
        qidx = np.zeros((NWIN, cap), np.int64)
        ea_t = np.zeros((P, NWIN * ntile), np.float32)
        S_all = np.zeros((P, NWIN * ntile * P), bf16)
        for w in range(NWIN):
            g = c * NWIN + w
            lo, hi = starts[g], starts[g + 1]
            n = hi - lo
            kidx[w, :n] = src_s[lo:hi]
            qidx[w, :n] = (dst_s[lo:hi] - g * WIN) + w * WIN
            qidx[w, n:] = w * WIN  # pad: gather a real local row
            d_pad = np.full(cap, -1, np.int64)
            d_pad[:n] = dst_s[lo:hi] - g * WIN
            e_pad = np.zeros(cap, np.float32)
            e_pad[:n] = ea_s[lo:hi]
            for j in range(ntile):
                sl = slice(j * P, (j + 1) * P)
                col = w * ntile + j
                ea_t[:, col] = e_pad[sl]
                dj = d_pad[sl]
                valid = dj >= 0
                Sb = np.zeros((P, P), np.float32)
                Sb[np.arange(P)[valid], dj[valid]] = 1.0
                S_all[:, col * P:(col + 1) * P] = Sb.astype(bf16)
        kidx_w = np.concatenate([_wrap_idx(kidx[w]) for w in range(NWIN)], axis=1)
        qidx_w = np.concatenate([_wrap_idx(qidx[w]) for w in range(NWIN)], axis=1)
        m = dict(shared)
        m["xT_bf"] = np.ascontiguousarray(xT[:, c * NSL:(c + 1) * NSL]).astype(bf16)
        m["kidx"] = kidx_w
        m["qidx"] = qidx_w
        m["ea_t"] = ea_t
        m["S_all"] = S_all
        m["xtT_bf"] = np.ascontiguousarray(
            xt_emb[c * NTL:(c + 1) * NTL].T).astype(bf16)
        per_core.append(m)
    return per_core, ntile


# --------------------------------------------------------------------------
# program builder
# --------------------------------------------------------------------------

def _build(ntile, debug=False):
    no_tp = os.environ.get("KB_NOTP", "0") == "1"      # disable tile_position
    no_bc = os.environ.get("KB_NOBC", "0") == "1"      # no ACT broadcast-expand
    no_acc = os.environ.get("KB_NOACC", "0") == "1"    # no ACT Copy+accum
    nc = bacc.Bacc("TRN2", target_bir_lowering=False, debug=False, num_devices=M)
    b16 = dt.bfloat16
    f32 = dt.float32
    cap = ntile * P
    IC = cap // 16                   # idx cols per window
    h0 = (ntile + 1) // 2            # half-window tile counts
    h1 = ntile - h0
    halves = [(0, h0), (h0, h1)]

    # ---- I/O ----
    xT_bf = nc.dram_tensor("xT_bf", [NS, NSL], b16, kind="ExternalInput")
    W1 = nc.dram_tensor("W1", [NS, 260], b16, kind="ExternalInput")
    W2 = nc.dram_tensor("W2", [NS, 256], b16, kind="ExternalInput")
    B1 = nc.dram_tensor("B1", [1, 260], b16, kind="ExternalInput")
    B2 = nc.dram_tensor("B2", [1, 256], b16, kind="ExternalInput")
    S_all = nc.dram_tensor("S_all", [P, NWIN * ntile * P], b16, kind="ExternalInput")
    kidx = nc.dram_tensor("kidx", [P, NWIN * IC], dt.int16, kind="ExternalInput")
    qidx = nc.dram_tensor("qidx", [P, NWIN * IC], dt.int16, kind="ExternalInput")
    ea_t = nc.dram_tensor("ea_t", [P, NWIN * ntile], f32, kind="ExternalInput")
    We_row = nc.dram_tensor("We_row", [1, D], f32, kind="ExternalInput")
    xtT_in = nc.dram_tensor("xtT_bf", [D, NTL], b16, kind="ExternalInput")
    q2w_in = nc.dram_tensor("q2w", [D, D], b16, kind="ExternalInput")
    q2b_in = nc.dram_tensor("q2b", [1, D], b16, kind="ExternalInput")
    k2w_in = nc.dram_tensor("k2w", [D, D], b16, kind="ExternalInput")
    k2b_in = nc.dram_tensor("k2b", [1, D], b16, kind="ExternalInput")
    sk2w_in = nc.dram_tensor("sk2w", [D, D], b16, kind="ExternalInput")
    sk2b_in = nc.dram_tensor("sk2b", [1, D], b16, kind="ExternalInput")
    v2wa_in = nc.dram_tensor("v2wa", [D, 132], b16, kind="ExternalInput")
    v2ba_in = nc.dram_tensor("v2ba", [1, 132], b16, kind="ExternalInput")
    gn1_in = nc.dram_tensor("gn1_cols", [D, 3], f32, kind="ExternalInput")
    gn2_in = nc.dram_tensor("gn2_cols", [D, 3], f32, kind="ExternalInput")
    ones_in = nc.dram_tensor("ones_bf", [1, 512], b16, kind="ExternalInput")

    adj_out = nc.dram_tensor("adj_out", [NTL, NT], f32, kind="ExternalOutput")
    if debug:
        dbg_kv = nc.dram_tensor("dbg_kv", [NSL, 256], b16, kind="ExternalOutput")
        dbg_hT = nc.dram_tensor("dbg_hT", [P, NSL], b16, kind="ExternalOutput")
        dbg_xtp = nc.dram_tensor("dbg_xtp", [P, NTL], f32, kind="ExternalOutput")

    # internal DRAM
    kv_loc = nc.dram_tensor("kv_loc", [NSL, 256], b16)
    kv_full = nc.dram_tensor("kv_full", [NS, 256], b16, addr_space="Shared")
    q_loc = nc.dram_tensor("q_loc", [NSL, 256], b16)
    hT_loc = nc.dram_tensor("hT_loc", [P, NSL], b16)
    hT_stack = nc.dram_tensor("hT_stack", [M * P, NSL], b16, addr_space="Shared")
    st_loc = nc.dram_tensor("st_loc", [P, 8], f32)
    st_full = nc.dram_tensor("st_full", [P, 8], f32, addr_space="Shared")
    xtn_loc = nc.dram_tensor("xtn_loc", [P, NTL], b16)
    xtn_stack = nc.dram_tensor("xtn_stack", [M * P, NTL], b16, addr_space="Shared")
    mm_loc = nc.dram_tensor("mm_loc", [1, 8], f32)
    mm_full = nc.dram_tensor("mm_full", [1, 8], f32, addr_space="Shared")

    rg = [list(range(M))]

    with tile.TileContext(nc) as tc:
        with (
            tc.tile_pool(name="persist", bufs=1) as pp,
        ):
            # ---- persistent small tiles ----
            ea_sb = pp.tile([P, NWIN * ntile], f32, tag="ea")
            nc.sync.dma_start(ea_sb[:], ea_t[:])
            kidx_sb = pp.tile([P, NWIN * IC], dt.int16, tag="kidx")
            nc.sync.dma_start(kidx_sb[:], kidx[:])
            qidx_sb = pp.tile([P, NWIN * IC], dt.int16, tag="qidx")
            nc.sync.dma_start(qidx_sb[:], qidx[:])
            ones_sb = pp.tile([1, 512], b16, tag="ones")
            nc.sync.dma_start(ones_sb[:], ones_in[:])
            We_sb = pp.tile([1, D], f32, tag="We")
            nc.sync.dma_start(We_sb[:], We_row[:])
            gn1_sb = pp.tile([D, 3], f32, tag="gn1")
            nc.sync.dma_start(gn1_sb[:], gn1_in[:])
            gn2_sb = pp.tile([D, 3], f32, tag="gn2")
            nc.sync.dma_start(gn2_sb[:], gn2_in[:])
            ident = pp.tile([P, P], f32, tag="ident")
            make_identity(nc, ident)
            ones_f32_row = pp.tile([1, P], f32, tag="ones_f32")
            nc.vector.memset(ones_f32_row[:], 1.0)
            skip_sb = pp.tile([P, NWIN * D], f32, tag="skip1")
            S_sb = pp.tile([P, NWIN * ntile * P], b16, tag="S_all")
            nc.sync.dma_start(S_sb[:], S_all[:])
            hT_lsb = pp.tile([P, NSL], b16, tag="hT_lsb")
            # We replicated [128,128] f32
            We_rep = pp.tile([P, P], f32, tag="We_rep")

            # stage-2 persistent
            xtT_sb = pp.tile([D, NTL], b16, tag="xtT")
            nc.sync.dma_start(xtT_sb[:], xtT_in[:])
            k2T_sb = pp.tile([D, NS], b16, tag="k2T")
            q2T_sb = pp.tile([D, NTL], b16, tag="q2T")
            skT_sb = pp.tile([D, NTL], f32, tag="skT")
            hrelu = pp.tile([D, NS], b16, tag="hrelu")
            hrelu_loc = pp.tile([D, NSL], b16, tag="hrelu_loc")

            # ================= P1 =================
            with (
                tc.tile_pool(name="p1w", bufs=1) as wp,
                tc.tile_pool(name="p1sb", bufs=3) as p1,
                tc.tile_pool(name="p1ps", bufs=2, space="PSUM") as p1ps,
                tc.tile_pool(name="p1ps2", bufs=2, space="PSUM") as p1ps2,
            ):
                xres = wp.tile([P, 32 * NSL], b16, tag="xres")
                nc.sync.dma_start(
                    xres[:].rearrange("p (k n) -> p k n", k=32),
                    xT_bf[:].rearrange("(k p) n -> p k n", p=P))
                W2_sb = wp.tile([P, 32 * 256], b16, tag="W2")
                nc.sync.dma_start(
                    W2_sb[:].rearrange("p (k n) -> p k n", k=32),
                    W2[:].rearrange("(k p) n -> p k n", p=P))
                W1_sb = wp.tile([P, 32 * 260], b16, tag="W1")
                nc.sync.dma_start(
                    W1_sb[:].rearrange("p (k n) -> p k n", k=32),
                    W1[:].rearrange("(k p) n -> p k n", p=P))
                B1_sb = wp.tile([1, 260], b16, tag="B1")
                nc.sync.dma_start(B1_sb[:], B1[:])
                B2_sb = wp.tile([1, 256], b16, tag="B2")
                nc.sync.dma_start(B2_sb[:], B2[:])

                # We_rep = ones_col.T @ We_row
                wr_ps = p1ps.tile([P, 256], f32, space="PSUM", tag="ps")
                nc.tensor.matmul(wr_ps[:, :P], ones_f32_row[:], We_sb[:],
                                 start=True, stop=True)
                nc.vector.tensor_copy(We_rep[:], wr_ps[:, :P])

                # ---- P1-A: kv ----
                for mt in range(NWIN):
                    ps2 = p1ps.tile([P, 256], f32, space="PSUM", tag="ps")
                    nc.tensor.matmul(ps2[:], ones_sb[:, :P], B2_sb[:],
                                     start=True, stop=False)
                    for kt in range(32):
                        nc.tensor.matmul(
                            ps2[:],
                            xres[:, kt * NSL + mt * P:kt * NSL + (mt + 1) * P],
                            W2_sb[:, kt * 256:(kt + 1) * 256],
                            start=False, stop=(kt == 31))
                    kv_st = p1.tile([P, 256], b16, tag="kvst")
                    nc.vector.tensor_copy(kv_st[:], ps2[:])
                    nc.sync.dma_start(kv_loc[mt * P:(mt + 1) * P, :], kv_st[:])

                nc.gpsimd.collective_compute(
                    "AllGather", OP.bypass,
                    ins=[kv_loc[:]], outs=[kv_full[:]], replica_groups=rg)

                # ---- P1-B: qq / skip ----
                for mt in range(NWIN):
                    ps1 = p1ps2.tile([P, 260], f32, space="PSUM", tag="ps1")
                    nc.tensor.matmul(ps1[:], ones_sb[:, :P], B1_sb[:],
                                     start=True, stop=False)
                    for kt in range(32):
                        nc.tensor.matmul(
                            ps1[:],
                            xres[:, kt * NSL + mt * P:kt * NSL + (mt + 1) * P],
                            W1_sb[:, kt * 260:(kt + 1) * 260],
                            start=False, stop=(kt == 31))
                    q_st = p1.tile([P, 256], b16, tag="qst")
                    nc.vector.memset(q_st[:], 0.0)
                    nc.vector.tensor_copy(q_st[:, 0:132], ps1[:, 0:132])
                    nc.sync.dma_start(q_loc[mt * P:(mt + 1) * P, :], q_st[:])
                    nc.vector.tensor_copy(skip_sb[:, mt * D:(mt + 1) * D],
                                          ps1[:, 132:260])
                if debug:
                    dkv = p1.tile([P, 256], b16, tag="dkvf")
                    for mt in range(NWIN):
                        nc.sync.dma_start(dkv[:], kv_loc[mt * P:(mt + 1) * P, :])
                        nc.sync.dma_start(dbg_kv[mt * P:(mt + 1) * P, :], dkv[:])

            # ================= edge phase =================
            with (
                tc.tile_pool(name="egath", bufs=2) as eg,
                tc.tile_pool(name="escr", bufs=2) as es,
                tc.tile_pool(name="erhs", bufs=2) as er,
                tc.tile_pool(name="eaggps", bufs=2, space="PSUM") as aps,
                tc.tile_pool(name="etrps", bufs=2, space="PSUM") as tps,
            ):
                for w in range(NWIN):
                    gkv = eg.tile([P, ntile * 256], b16, tag="gkv")
                    nc.gpsimd.dma_gather(
                        out_ap=gkv[:].rearrange("p (t e) -> p t e", e=256),
                        in_ap=kv_full[:],
                        idxs_ap=kidx_sb[:, w * IC:(w + 1) * IC],
                        num_idxs=cap, num_idxs_reg=cap, elem_size=256,
                        single_packet=False)
                    gq = eg.tile([P, ntile * 256], b16, tag="gq")
                    nc.gpsimd.dma_gather(
                        out_ap=gq[:].rearrange("p (t e) -> p t e", e=256),
                        in_ap=q_loc[:],
                        idxs_ap=qidx_sb[:, w * IC:(w + 1) * IC],
                        num_idxs=cap, num_idxs_reg=cap, elem_size=256,
                        single_packet=False)

                    rhs = er.tile([P, ntile * 136], b16, tag="rhs")
                    rhs3 = rhs[:].rearrange("p (t x) -> p t x", x=136)
                    agg_ps = aps.tile([P, 136], f32, space="PSUM", tag="agg")

                    for hb, (j0, nj) in enumerate(halves):
                        gkv3 = gkv[:].rearrange("p (t e) -> p t e", e=256)
                        gq3 = gq[:].rearrange("p (t e) -> p t e", e=256)
                        sl = slice(j0, j0 + nj)
                        qk = es.tile([P, nj * D], b16, tag=f"qk{hb}")
                        nc.vector.tensor_tensor(
                            out=qk[:].rearrange("p (t e) -> p t e", e=D),
                            in0=gq3[:, sl, 0:D], in1=gkv3[:, sl, 0:D],
                            op=OP.mult)
                        alpha = es.tile([P, nj * H], f32, tag=f"al{hb}")
                        nc.vector.reduce_sum(
                            out=alpha[:],
                            in_=qk[:].rearrange("p (g c) -> p g c", c=C),
                            axis=AX.X)
                        ea3 = ea_sb[:, w * ntile + j0:w * ntile + j0 + nj]
                        awe = es.tile([P, nj * H], f32, tag=f"awe{hb}")
                        nc.vector.tensor_tensor(
                            out=awe[:].rearrange("p (t h) -> p t h", h=H),
                            in0=gq3[:, sl, D:D + H],
                            in1=ea3.unsqueeze(2).to_broadcast([P, nj, H]),
                            op=OP.mult)
                        nc.vector.tensor_tensor(
                            out=alpha[:], in0=alpha[:], in1=awe[:], op=OP.add)
                        # ex -> rhs[:, t, 0:4]
                        nc.scalar.activation(
                            rhs3[:, sl, 0:H],
                            alpha[:].rearrange("p (t h) -> p t h", h=H),
                            AF.Exp)
                        # ex*ea -> rhs[:, t, 4:8]
                        nc.vector.tensor_tensor(
                            out=rhs3[:, sl, H:2 * H], in0=rhs3[:, sl, 0:H],
                            in1=ea3.unsqueeze(2).to_broadcast([P, nj, H]),
                            op=OP.mult)
                        # v*ex -> rhs[:, t, 8:136]
                        if no_bc:
                            nc.vector.tensor_tensor(
                                out=rhs3[:, sl, 2 * H:136].rearrange(
                                    "p t (h c) -> p t h c", h=H),
                                in0=gkv3[:, sl, D:256].rearrange(
                                    "p t (h c) -> p t h c", h=H),
                                in1=rhs3[:, sl, 0:H].unsqueeze(3)
                                    .to_broadcast([P, nj, H, C]),
                                op=OP.mult)
                        else:
                            # expand ex over channels on ACT, then 2x DVE mult
                            exx = es.tile([P, nj * D], b16, tag=f"exx{hb}")
                            nc.scalar.activation(
                                exx[:].rearrange("p (t h c) -> p t h c", h=H, c=C),
                                rhs3[:, sl, 0:H].unsqueeze(3)
                                    .to_broadcast([P, nj, H, C]),
                                AF.Copy)
                            nc.vector.tensor_tensor(
                                out=rhs3[:, sl, 2 * H:136],
                                in0=gkv3[:, sl, D:256],
                                in1=exx[:].rearrange("p (t e) -> p t e", e=D),
                                op=OP.mult)

                    for j in range(ntile):
                        nc.tensor.matmul(
                            agg_ps[:],
                            S_sb[:, (w * ntile + j) * P:(w * ntile + j + 1) * P],
                            rhs[:, j * 136:(j + 1) * 136],
                            start=(j == 0), stop=(j == ntile - 1))

                    # ---- finalize window ----
                    invd = es.tile([P, H], f32, tag="invd")
                    nc.vector.reciprocal(invd[:], agg_ps[:, 0:H])
                    hpre = es.tile([P, D], f32, tag="hpre")
                    nc.vector.tensor_tensor(
                        out=hpre[:].rearrange("p (h c) -> p h c", h=H),
                        in0=agg_ps[:, H:2 * H].unsqueeze(2).to_broadcast([P, H, C]),
                        in1=We_rep[:].rearrange("p (h c) -> p h c", h=H),
                        op=OP.mult)
                    nc.vector.tensor_tensor(
                        out=hpre[:], in0=hpre[:], in1=agg_ps[:, 2 * H:136],
                        op=OP.add)
                    nc.vector.tensor_tensor(
                        out=hpre[:].rearrange("p (h c) -> p h c", h=H),
                        in0=hpre[:].rearrange("p (h c) -> p h c", h=H),
                        in1=invd[:].unsqueeze(2).to_broadcast([P, H, C]),
                        op=OP.mult)
                    nc.vector.tensor_tensor(
                        out=hpre[:], in0=hpre[:],
                        in1=skip_sb[:, w * D:(w + 1) * D], op=OP.add)
                    tr_ps = tps.tile([P, P], f32, space="PSUM", tag="tr")
                    nc.tensor.transpose(tr_ps[:], hpre[:], ident[:])
                    nc.vector.tensor_copy(hT_lsb[:, w * P:(w + 1) * P], tr_ps[:])

                nc.sync.dma_start(hT_loc[:], hT_lsb[:])
                nc.gpsimd.collective_compute(
                    "AllGather", OP.bypass,
                    ins=[hT_loc[:]], outs=[hT_stack[:]], replica_groups=rg)

            # ================= gn1 =================
            with (
                tc.tile_pool(name="gsb", bufs=1) as gs,
                tc.tile_pool(name="gscr", bufs=2) as gsc,
            ):
                hT_pre = gs.tile([P, NS], b16, tag="hT_pre")
                for r in range(M):
                    nc.sync.dma_start(hT_pre[:, r * NSL:(r + 1) * NSL],
                                      hT_stack[r * P:(r + 1) * P, :])
                ssum = gsc.tile([P, 1], f32, tag="ssum")
                scr = gsc.tile([P, NS], b16, tag="scr")
                if no_acc:
                    nc.vector.reduce_sum(out=ssum[:], in_=hT_pre[:], axis=AX.X)
                else:
                    nc.scalar.activation(scr[:], hT_pre[:], AF.Copy,
                                         accum_out=ssum[:])
                mean = gsc.tile([P, 4], f32, tag="mean")
                nc.vector.tensor_scalar_mul(mean[:, 0:1], ssum[:], float(1.0 / NS))
                # msmean = mean*ms
                nc.vector.tensor_tensor(out=mean[:, 1:2], in0=mean[:, 0:1],
                                        in1=gn1_sb[:, 2:3], op=OP.mult)
                xc = gs.tile([P, NS], b16, tag="xc")
                nc.vector.tensor_scalar_sub(xc[:], hT_pre[:], mean[:, 1:2])
                sumsq = gsc.tile([P, 1], f32, tag="sumsq")
                nc.scalar.activation(scr[:], xc[:], AF.Square, accum_out=sumsq[:])
                var = gsc.tile([P, 1], f32, tag="var")
                nc.vector.tensor_scalar(
                    out=var[:], in0=sumsq[:], scalar1=float(1.0 / NS),
                    scalar2=float(EPS_GN), op0=OP.mult, op1=OP.add)
                nc.scalar.sqrt(var[:], var[:])
                rstd = gsc.tile([P, 1], f32, tag="rstd")
                nc.vector.reciprocal(rstd[:], var[:])
                scale1 = gsc.tile([P, 1], f32, tag="scale1")
                nc.vector.tensor_tensor(out=scale1[:], in0=gn1_sb[:, 0:1],
                                        in1=rstd[:], op=OP.mult)
                nc.scalar.activation(hrelu[:], xc[:], AF.Relu,
                                     bias=gn1_sb[:, 1:2], scale=scale1[:, 0:1])
                # local slice (pre-AG values) for gn2 stats
                xcl = gsc.tile([P, NSL], b16, tag="xcl")
                nc.vector.tensor_scalar_sub(xcl[:], hT_lsb[:], mean[:, 1:2])
                nc.scalar.activation(hrelu_loc[:], xcl[:], AF.Relu,
                                     bias=gn1_sb[:, 1:2], scale=scale1[:, 0:1])
            if debug:
                nc.sync.dma_start(dbg_hT[:], hT_lsb[:])

            # ================= stage 2 =================
            with (
                tc.tile_pool(name="s2w", bufs=1) as s2w,
                tc.tile_pool(name="s2ps", bufs=2, space="PSUM") as s2ps,
                tc.tile_pool(name="s2sc", bufs=2) as s2c,
            ):
                k2w_sb = s2w.tile([D, D], b16, tag="k2w")
                nc.sync.dma_start(k2w_sb[:], k2w_in[:])
                k2b_sb = s2w.tile([1, D], b16, tag="k2b")
                nc.sync.dma_start(k2b_sb[:], k2b_in[:])
                q2w_sb = s2w.tile([D, D], b16, tag="q2w")
                nc.sync.dma_start(q2w_sb[:], q2w_in[:])
                q2b_sb = s2w.tile([1, D], b16, tag="q2b")
                nc.sync.dma_start(q2b_sb[:], q2b_in[:])
                sk2w_sb = s2w.tile([D, D], b16, tag="sk2w")
                nc.sync.dma_start(sk2w_sb[:], sk2w_in[:])
                sk2b_sb = s2w.tile([1, D], b16, tag="sk2b")
                nc.sync.dma_start(sk2b_sb[:], sk2b_in[:])
                v2wa_sb = s2w.tile([D, 132], b16, tag="v2wa")
                nc.sync.dma_start(v2wa_sb[:], v2wa_in[:])
                v2ba_sb = s2w.tile([1, 132], b16, tag="v2ba")
                nc.sync.dma_start(v2ba_sb[:], v2ba_in[:])

                # q2T / skipT (can run early; only need inputs)
                qps = s2ps.tile([P, 512], f32, space="PSUM", tag="ps")
                nc.tensor.matmul(qps[:, :NTL], q2b_sb[:], ones_sb[:, :NTL],
                                 start=True, stop=False)
                nc.tensor.matmul(qps[:, :NTL], q2w_sb[:], xtT_sb[:],
                                 start=False, stop=True)
                nc.vector.tensor_copy(q2T_sb[:], qps[:, :NTL])
                # zero-masked q2T: head h's values live only at partitions
                # 32h:32h+32 of column block h => scores for all 4 heads via
                # ONE K=128 matmul per source chunk (no PE row-groups).
                q2z_sb = s2w.tile([D, H * NTL], b16, tag="q2z")
                nc.vector.memset(q2z_sb[:], 0.0)
                for h in range(H):
                    nc.vector.tensor_copy(
                        q2z_sb[32 * h:32 * (h + 1), h * NTL:(h + 1) * NTL],
                        q2T_sb[32 * h:32 * (h + 1), :])
                sps = s2ps.tile([P, 512], f32, space="PSUM", tag="ps")
                nc.tensor.matmul(sps[:, :NTL], sk2b_sb[:], ones_sb[:, :NTL],
                                 start=True, stop=False)
                nc.tensor.matmul(sps[:, :NTL], sk2w_sb[:], xtT_sb[:],
                                 start=False, stop=True)
                nc.vector.tensor_copy(skT_sb[:], sps[:, :NTL])

                # k2T over all sources
                for ch in range(8):
                    kps = s2ps.tile([P, 512], f32, space="PSUM", tag="ps")
                    nc.tensor.matmul(kps[:], k2b_sb[:], ones_sb[:],
                                     start=True, stop=False)
                    nc.tensor.matmul(kps[:], k2w_sb[:],
                                     hrelu[:, ch * 512:(ch + 1) * 512],
                                     start=False, stop=True)
                    if ch % 2 == 0:
                        nc.scalar.activation(
                            k2T_sb[:, ch * 512:(ch + 1) * 512], kps[:], AF.Copy)
                    else:
                        nc.vector.tensor_copy(
                            k2T_sb[:, ch * 512:(ch + 1) * 512], kps[:])

                # gn2 source-part stats: skip2T over own nodes
                st_sb = s2c.tile([P, 8], f32, tag="st")
                nc.vector.memset(st_sb[:], 0.0)
                scr2 = s2c.tile([P, 512], b16, tag="scr2")
                gps = s2ps.tile([P, 512], f32, space="PSUM", tag="ps")
                nc.tensor.matmul(gps[:], sk2b_sb[:], ones_sb[:],
                                 start=True, stop=False)
                nc.tensor.matmul(gps[:], sk2w_sb[:], hrelu_loc[:],
                                 start=False, stop=True)
                nc.vector.reduce_sum(out=st_sb[:, 0:1], in_=gps[:], axis=AX.X)
                nc.scalar.activation(scr2[:], gps[:], AF.Square,
                                     accum_out=st_sb[:, 1:2])

                # v2a for all source chunks (upfront; frees PSUM for the s-loop)
                v2a_all = s2w.tile([P, 32 * 132], b16, tag="v2a_all")
                for st in range(32):
                    vps = s2ps.tile([P, 512], f32, space="PSUM", tag="ps")
                    nc.tensor.matmul(vps[:, :132], ones_sb[:, :P], v2ba_sb[:],
                                     start=True, stop=False)
                    nc.tensor.matmul(vps[:, :132],
                                     hrelu[:, st * P:(st + 1) * P],
                                     v2wa_sb[:], start=False, stop=True)
                    if st % 2 == 0:
                        nc.scalar.activation(
                            v2a_all[:, st * 132:(st + 1) * 132], vps[:, :132],
                            AF.Copy)
                    else:
                        nc.vector.tensor_copy(
                            v2a_all[:, st * 132:(st + 1) * 132], vps[:, :132])

                # ---- attention s-loop ----
                with (
                    tc.tile_pool(name="scps", bufs=2, space="PSUM") as scp,
                    tc.tile_pool(name="ndps", bufs=1, space="PSUM") as ndp,
                    tc.tile_pool(name="sexp", bufs=3) as sxp,
                ):
                    nd_ps = ndp.tile([33, H * NTL], f32, space="PSUM", tag="nd")
                    for st in range(32):
                        scps = scp.tile([P, H * NTL], f32, space="PSUM", tag="sc")
                        for half in range(2):
                            nc.tensor.matmul(
                                scps[:, half * 512:(half + 1) * 512],
                                k2T_sb[:, st * P:(st + 1) * P],
                                q2z_sb[:, half * 512:(half + 1) * 512],
                                start=True, stop=True)
                        exs = sxp.tile([P, H * NTL], b16, tag="exs")
                        nc.scalar.activation(exs[:], scps[:], AF.Exp)
                        for h in range(H):
                            nc.tensor.matmul(
                                nd_ps[:, h * NTL:(h + 1) * NTL],
                                v2a_all[:, st * 132 + 33 * h:st * 132 + 33 * (h + 1)],
                                exs[:, h * NTL:(h + 1) * NTL],
                                start=(st == 0), stop=(st == 31))

                    # xtpT = nd/denom + skT
                    xtpT = s2c.tile([D, NTL], f32, tag="xtpT")
                    for h in range(H):
                        denrow = s2c.tile([1, NTL], f32, tag="denrow")
                        nc.vector.tensor_copy(denrow[:],
                                              nd_ps[32:33, h * NTL:(h + 1) * NTL])
                        drep = s2ps.tile([P, 512], f32, space="PSUM", tag="ps")
                        nc.tensor.matmul(drep[:, :NTL], ones_f32_row[:],
                                         denrow[:], start=True, stop=True)
                        invd2 = s2c.tile([32, NTL], f32, tag="invd2")
                        nc.vector.reciprocal(invd2[:], drep[:32, :NTL])
                        nc.vector.tensor_tensor(
                            out=xtpT[32 * h:32 * (h + 1), :],
                            in0=nd_ps[0:32, h * NTL:(h + 1) * NTL],
                            in1=invd2[:], op=OP.mult)
                    nc.vector.tensor_tensor(out=xtpT[:], in0=xtpT[:],
                                            in1=skT_sb[:], op=OP.add)
                if debug:
                    nc.sync.dma_start(dbg_xtp[:], xtpT[:])

                # gn2 target-part stats + AllReduce
                tsum = s2c.tile([P, 2], f32, tag="tsum")
                nc.vector.reduce_sum(out=tsum[:, 0:1], in_=xtpT[:], axis=AX.X)
                scr3 = s2c.tile([P, NTL], b16, tag="scr3")
                nc.scalar.activation(scr3[:], xtpT[:], AF.Square,
                                     accum_out=tsum[:, 1:2])
                nc.vector.tensor_tensor(out=st_sb[:, 0:2], in0=st_sb[:, 0:2],
                                        in1=tsum[:], op=OP.add)
                nc.sync.dma_start(st_loc[:], st_sb[:])
                nc.gpsimd.collective_compute(
                    "AllReduce", OP.add,
                    ins=[st_loc[:]], outs=[st_full[:]], replica_groups=rg)
                stf = s2c.tile([P, 8], f32, tag="stf")
                nc.sync.dma_start(stf[:], st_full[:])

                NALL = float(NS + NT)
                t2 = s2c.tile([P, 6], f32, tag="t2")
                mean2 = t2[:, 0:1]
                nc.vector.tensor_scalar_mul(mean2, stf[:, 0:1], float(1.0 / NALL))
                # ms*(2-ms)
                nc.vector.tensor_scalar(
                    out=t2[:, 1:2], in0=gn2_sb[:, 2:3], scalar1=-1.0, scalar2=2.0,
                    op0=OP.mult, op1=OP.add)
                nc.vector.tensor_tensor(out=t2[:, 1:2], in0=t2[:, 1:2],
                                        in1=gn2_sb[:, 2:3], op=OP.mult)
                nc.vector.tensor_tensor(out=t2[:, 2:3], in0=mean2, in1=mean2,
                                        op=OP.mult)
                nc.vector.tensor_tensor(out=t2[:, 2:3], in0=t2[:, 2:3],
                                        in1=t2[:, 1:2], op=OP.mult)
                var2 = t2[:, 3:4]
                nc.vector.tensor_scalar_mul(var2, stf[:, 1:2], float(1.0 / NALL))
                nc.vector.tensor_tensor(out=var2, in0=var2, in1=t2[:, 2:3],
                                        op=OP.subtract)
                nc.vector.tensor_scalar_add(var2, var2, float(EPS_GN))
                nc.scalar.sqrt(var2, var2)
                rstd2 = t2[:, 4:5]
                nc.vector.reciprocal(rstd2, var2)
                scale2 = s2c.tile([P, 2], f32, tag="sc2")
                nc.vector.tensor_tensor(out=scale2[:, 0:1], in0=gn2_sb[:, 0:1],
                                        in1=rstd2, op=OP.mult)
                # bias2 = b - mean*ms*scale
                nc.vector.tensor_tensor(out=t2[:, 5:6], in0=mean2,
                                        in1=gn2_sb[:, 2:3], op=OP.mult)
                nc.vector.tensor_tensor(out=t2[:, 5:6], in0=t2[:, 5:6],
                                        in1=scale2[:, 0:1], op=OP.mult)
                nc.vector.tensor_scalar_mul(t2[:, 5:6], t2[:, 5:6], -1.0)
                nc.vector.tensor_tensor(out=scale2[:, 1:2], in0=gn2_sb[:, 1:2],
                                        in1=t2[:, 5:6], op=OP.add)

                xtnT = s2c.tile([D, NTL], b16, tag="xtnT")
                nc.scalar.activation(xtnT[:], xtpT[:], AF.Relu,
                                     bias=scale2[:, 1:2], scale=scale2[:, 0:1])
                nc.sync.dma_start(xtn_loc[:], xtnT[:])
                nc.gpsimd.collective_compute(
                    "AllGather", OP.bypass,
                    ins=[xtn_loc[:]], outs=[xtn_stack[:]], replica_groups=rg)

                # ---- adj ----
                xtfT = s2w.tile([D, NT], b16, tag="xtfT")
                for r in range(M):
                    nc.sync.dma_start(xtfT[:, r * NTL:(r + 1) * NTL],
                                      xtn_stack[r * P:(r + 1) * P, :])
                adj_sb = s2w.tile([P, 2 * NT], f32, tag="adj")
                mxc = s2c.tile([P, 2], f32, tag="mxc")
                first = True
                for mt in range(2):
                    for nk in range(4):
                        adps = s2ps.tile([P, 512], f32, space="PSUM", tag="ps")
                        nc.tensor.matmul(
                            adps[:],
                            xtnT[:, mt * P:(mt + 1) * P],
                            xtfT[:, nk * 512:(nk + 1) * 512],
                            start=True, stop=True)
                        nc.vector.tensor_copy(
                            adj_sb[:, mt * NT + nk * 512:mt * NT + (nk + 1) * 512],
                            adps[:])
                        tmx = s2c.tile([P, 2], f32, tag="tmx")
                        nc.vector.reduce_max(out=tmx[:, 0:1], in_=adps[:],
                                             axis=AX.X)
                        nc.vector.tensor_reduce(
                            out=tmx[:, 1:2], in_=adps[:], op=OP.min, axis=AX.X)
                        if first:
                            nc.vector.tensor_copy(mxc[:], tmx[:])
                            first = False
                        else:
                            nc.vector.tensor_tensor(
                                out=mxc[:, 0:1], in0=mxc[:, 0:1],
                                in1=tmx[:, 0:1], op=OP.max)
                            nc.vector.tensor_tensor(
                                out=mxc[:, 1:2], in0=mxc[:, 1:2],
                                in1=tmx[:, 1:2], op=OP.min)
                nc.vector.tensor_scalar_mul(mxc[:, 1:2], mxc[:, 1:2], -1.0)
                mxt_ps = s2ps.tile([P, 512], f32, space="PSUM", tag="ps")
                nc.tensor.transpose(mxt_ps[:2, :P], mxc[:], ident[:])
                mxrow = s2c.tile([2, P], f32, tag="mxrow")
                nc.vector.tensor_copy(mxrow[:], mxt_ps[:2, :P])
                mm2 = s2c.tile([2, 1], f32, tag="mm2")
                nc.vector.reduce_max(out=mm2[:], in_=mxrow[:], axis=AX.X)
                mm2t = s2ps.tile([P, 512], f32, space="PSUM", tag="ps")
                nc.tensor.transpose(mm2t[:1, :2], mm2[:], ident[:2, :2])
                mmrow = s2c.tile([1, 8], f32, tag="mmrow")
                nc.vector.memset(mmrow[:], -1e30)
                nc.vector.tensor_copy(mmrow[:, 0:2], mm2t[:1, :2])
                nc.sync.dma_start(mm_loc[:], mmrow[:])
                nc.gpsimd.collective_compute(
                    "AllReduce", OP.max,
                    ins=[mm_loc[:]], outs=[mm_full[:]], replica_groups=rg)
                mmf = s2c.tile([1, 8], f32, tag="mmf")
                nc.sync.dma_start(mmf[:], mm_full[:])
                sc = s2c.tile([1, 4], f32, tag="scl")
                nc.vector.tensor_tensor(out=sc[:, 0:1], in0=mmf[:, 0:1],
                                        in1=mmf[:, 1:2], op=OP.add)
                nc.vector.tensor_scalar_add(sc[:, 0:1], sc[:, 0:1], 1e-8)
                nc.vector.reciprocal(sc[:, 1:2], sc[:, 0:1])
                nc.vector.tensor_scalar_mul(sc[:, 2:3], mmf[:, 1:2], -1.0)
                mnrep = s2ps.tile([P, 512], f32, space="PSUM", tag="ps")
                nc.tensor.matmul(mnrep[:, :2], ones_f32_row[:], sc[:, 1:3],
                                 start=True, stop=True)
                mncol = s2c.tile([P, 2], f32, tag="mncol")
                nc.vector.tensor_copy(mncol[:], mnrep[:, :2])
                for mt in range(2):
                    onorm = s2w.tile([P, NT], f32, tag="onorm")
                    nc.vector.tensor_scalar(
                        out=onorm[:], in0=adj_sb[:, mt * NT:(mt + 1) * NT],
                        scalar1=mncol[:, 1:2], scalar2=mncol[:, 0:1],
                        op0=OP.subtract, op1=OP.mult)
                    nc.sync.dma_start(adj_out[mt * P:(mt + 1) * P, :], onorm[:])

    nc.compile()
    return nc


def _get_prog(ntile, debug=False):
    key = (ntile, debug)
    if key not in _prog_cache:
        _prog_cache[key] = _build(ntile, debug)
    return _prog_cache[key]


def kernel(**inputs):
    per_core, ntile = _prep(inputs)
    debug = os.environ.get("KB_DEBUG", "0") == "1"
    nc = _get_prog(ntile, debug)
    trace = os.environ.get("KB_TRACE", "0") == "1"
    res = run_bass_kernel_spmd(nc, per_core, core_ids=list(range(M)), trace=trace)
    if trace:
        kernel.last_result = res
    out = np.concatenate([res.results[c]["adj_out"] for c in range(M)], axis=0)
    if debug:
        kernel.debug_results = res.results
    return out


# revision 29
# speedup vs baseline: 1.3445x; 1.3445x over previous
"""Trainium2 Bass kernel for nn_BiMP (GNN message passing), 8 NeuronCores SPMD.

v3. Per-core pipeline (all matmuls bf16, fp32 PSUM):
  P1-A: kv = x_c @ [k1|v1] + b  -> kv_loc -> AllGather -> kv_sb (SBUF resident)
  P1-B: qq = x_c @ [q1*isq | qWe | skip1] + b -> qq_sb / skip_sb (SBUF)
  edge phase (dst-window = 128 dst nodes; within a window edges are grouped
  by src-block with canonical per-(window, block) slot budgets = max count
  over cores, so the one-hot matmul schedule is identical across cores):
    gkv[slot] = sum_b E_(tile,b).T @ kv_sb[block b]   (one-hot gather-by-matmul)
    qexp[slot] = ST_tile.T @ qq_w                     (one-hot expand)
    alpha = sum_c q*k per head + ea*qWe (DVE, half-window batches, bf16 2x)
    ex = exp(alpha); rhs = [ex | ex*ea | v*ex]; agg += S_tile.T @ rhs_tile
    h = (aggv + (sum ex*ea)*We)/denom + skip; transpose -> hT bf16 -> AllGather
  gn1 stats redundant per core; relu-normalize (ACT, feature-major)
  stage 2 (dense bipartite attention, feature-major):
    k2T = k2w.T @ hrelu; q2T zero-masked block-diagonal => scores for all 4
    heads via single K=128 matmuls; exp (ACT); nd_h += v2a_h.T @ exp_h;
    xtpT = nd/denom + skipT; gn2 stats AllReduce; xtnT = relu-normalize;
  AllGather xtnT -> adj = xtnT.T @ xtfT; minmax AllReduce; normalize.
Self-contained; compiles per edge-structure (compile time is not scored).
"""
import os
import sys
import types

import numpy as np


def _install_ntff_shim():
    """bass_utils imports antenv.axon_hooks when tracing; provide it."""
    if "antenv.axon_hooks" in sys.modules:
        return
    mod = types.ModuleType("antenv.axon_hooks")

    def set_axon_ntff_profile_hook(h):
        mod._hook = h

    def get_axon_ntff_profile_hook():
        return getattr(mod, "_hook", None)

    mod.set_axon_ntff_profile_hook = set_axon_ntff_profile_hook
    mod.get_axon_ntff_profile_hook = get_axon_ntff_profile_hook
    sys.modules["antenv.axon_hooks"] = mod
    try:
        import antenv
        antenv.axon_hooks = mod
        from trn_agent_boot.trn_boot import _ntff_profile_via_ctypes
        set_axon_ntff_profile_hook(_ntff_profile_via_ctypes("/opt/axon/libaxon_pjrt.so"))
    except Exception:
        pass


_install_ntff_shim()

import ml_dtypes
import concourse.bacc as bacc
import concourse.bass as bass
import concourse.mybir as mybir
import concourse.tile as tile
from concourse.bass_utils import run_bass_kernel_spmd
from concourse.masks import make_identity

dt = mybir.dt
bf16 = ml_dtypes.bfloat16
AF = mybir.ActivationFunctionType
OP = mybir.AluOpType
AX = mybir.AxisListType

NS, NT, H, C = 4096, 2048, 4, 32
D = H * C            # 128
E1 = 131072
M = 8                # cores
NSL = NS // M        # 512 source nodes / core
NTL = NT // M        # 256 target rows / core
WIN = 128            # dst nodes per window
NWIN = NSL // WIN    # 4 windows / core
P = 128
NBLK = NS // P       # 32 src blocks
ISQ = np.float32(1.0 / np.sqrt(np.float32(C)))
EPS_GN = np.float32(1e-5)

_prog_cache = {}


# --------------------------------------------------------------------------
# host-side preparation
# --------------------------------------------------------------------------

def _prep(inputs):
    f32 = lambda k: np.asarray(inputs[k], np.float32)
    x = f32("x")
    src = np.asarray(inputs["pos_edge_index"][0]).astype(np.int64)
    dst = np.asarray(inputs["pos_edge_index"][1]).astype(np.int64)
    ea = f32("edge_attr").reshape(-1)
    xt_emb = f32("target_node_embeddings")

    We = f32("e1_w").reshape(D)
    M2T = np.zeros((D, H), np.float32)
    for h in range(H):
        M2T[h * C:(h + 1) * C, h] = We[h * C:(h + 1) * C]
    Wq_s = f32("q1_w") * ISQ
    Wc = np.ascontiguousarray(np.concatenate(
        [Wq_s, f32("skip1_w"), f32("k1_w"), f32("v1_w")], axis=1)).astype(bf16)
    bq = f32("q1_b") * ISQ
    Bc = np.concatenate([bq, f32("skip1_b"), f32("k1_b"),
                         f32("v1_b")]).reshape(1, 512).astype(bf16)

    order = np.argsort(dst, kind="stable")
    src_s, dst_s, ea_s = src[order], dst[order], ea[order]
    win_id = dst_s // WIN
    counts = np.bincount(win_id, minlength=NS // WIN)
    starts = np.zeros(NS // WIN + 1, np.int64)
    np.cumsum(counts, out=starts[1:])

    # canonical per-(window, block) slot budgets = max count over cores
    cnt3 = np.zeros((M, NWIN, NBLK), np.int64)
    for c in range(M):
        for w in range(NWIN):
            g = c * NWIN + w
            lo, hi = starts[g], starts[g + 1]
            cnt3[c, w] = np.bincount(src_s[lo:hi] // P, minlength=NBLK)
    budget = cnt3.max(axis=0)                  # [NWIN, NBLK]
    cap_w, ntile_w, pairs_w, boff_w = [], [], [], []
    for w in range(NWIN):
        off = np.zeros(NBLK + 1, np.int64)
        np.cumsum(budget[w], out=off[1:])
        tot = int(off[-1])
        cap = max(int(np.ceil(tot / P) * P), P)
        ntw = cap // P
        pl = []
        for j in range(ntw):
            j0, j1 = j * P, (j + 1) * P
            for b in range(NBLK):
                if budget[w][b] > 0 and off[b] < j1 and off[b + 1] > j0:
                    pl.append((j, b))
        cap_w.append(cap)
        ntile_w.append(ntw)
        pairs_w.append(pl)
        boff_w.append(off)
    TT = sum(ntile_w)            # total tiles
    TP = sum(len(p) for p in pairs_w)  # total pairs

    shared = {
        "Wc": Wc, "Bc": Bc,
        "We_row": We.reshape(1, D).astype(np.float32),
        "q2w": np.ascontiguousarray(f32("q2_w") * ISQ).astype(bf16),
        "q2b": (f32("q2_b") * ISQ).reshape(1, D).astype(bf16),
        "k2w": f32("k2_w").astype(bf16),
        "k2b": f32("k2_b").reshape(1, D).astype(bf16),
        "k2bc": f32("k2_b").reshape(D, 1),
        "sk2w": f32("skip2_w").astype(bf16),
        "sk2b": f32("skip2_b").reshape(1, D).astype(bf16),
        "gn1_cols": np.stack([f32("gn1_w"), f32("gn1_b"), f32("gn1_ms")], axis=1),
        "gn2_cols": np.stack([f32("gn2_w"), f32("gn2_b"), f32("gn2_ms")], axis=1),
        "ones_bf": np.ones((1, 512), bf16),
    }
    v2wa = np.zeros((D, 4 * 33), np.float32)
    v2ba = np.zeros((1, 4 * 33), np.float32)
    v2w_np, v2b_np = f32("v2_w"), f32("v2_b")
    for h in range(H):
        v2wa[:, 33 * h:33 * h + 32] = v2w_np[:, 32 * h:32 * (h + 1)]
        v2ba[0, 33 * h:33 * h + 32] = v2b_np[32 * h:32 * (h + 1)]
        v2ba[0, 33 * h + 32] = 1.0
    shared["v2wa"] = v2wa.astype(bf16)
    shared["v2ba"] = v2ba.astype(bf16)

    xT = x.T
    per_core = []
    for c in range(M):
        ea_t = np.zeros((P, TT), np.float32)
        S_all = np.zeros((P, TT * P), bf16)
        ST_all = np.zeros((P, TT * P), bf16)
        E_all = np.zeros((P, TP * P), bf16)
        toff = 0
        poff = 0
        for w in range(NWIN):
            g = c * NWIN + w
            lo, hi = starts[g], starts[g + 1]
            cap, ntw, off = cap_w[w], ntile_w[w], boff_w[w]
            # place edges: block b's edges at slots [off[b], off[b]+count)
            sw, dw, eaw = src_s[lo:hi], dst_s[lo:hi] - g * WIN, ea_s[lo:hi]
            o2 = np.argsort(sw // P, kind="stable")
            sw, dw, eaw = sw[o2], dw[o2], eaw[o2]
            s_pad = np.full(cap, -1, np.int64)   # -1 = dead slot (zero E col)
            d_pad = np.full(cap, -1, np.int64)
            e_pad = np.zeros(cap, np.float32)
            pos = 0
            for b in range(NBLK):
                nb = cnt3[c, w, b]
                sl = slice(int(off[b]), int(off[b]) + int(nb))
                s_pad[sl] = sw[pos:pos + nb]
                d_pad[sl] = dw[pos:pos + nb]
                e_pad[sl] = eaw[pos:pos + nb]
                pos += nb
            for j in range(ntw):
                sl = slice(j * P, (j + 1) * P)
                col = toff + j
                ea_t[:, col] = e_pad[sl]
                dj = d_pad[sl]
                valid = dj >= 0
                Sb = np.zeros((P, P), np.float32)
                Sb[np.arange(P)[valid], dj[valid]] = 1.0
                S_all[:, col * P:(col + 1) * P] = Sb.astype(bf16)
                ST_all[:, col * P:(col + 1) * P] = Sb.T.astype(bf16)
            for i, (j, b) in enumerate(pairs_w[w]):
                sj = s_pad[j * P:(j + 1) * P]
                mk = (sj >= 0) & ((sj // P) == b)
                Eb = np.zeros((P, P), np.float32)
                Eb[sj[mk] - b * P, np.arange(P)[mk]] = 1.0
                E_all[:, (poff + i) * P:(poff + i + 1) * P] = Eb.astype(bf16)
            toff += ntw
            poff += len(pairs_w[w])
        m = dict(shared)
        m["xT_bf"] = np.ascontiguousarray(xT[:, c * NSL:(c + 1) * NSL]).astype(bf16)
        m["ea_t"] = ea_t
        m["S_all"] = S_all
        m["ST_all"] = ST_all
        m["E_all"] = E_all
        m["xtT_bf"] = np.ascontiguousarray(
            xt_emb[c * NTL:(c + 1) * NTL].T).astype(bf16)
        per_core.append(m)
    struct = (tuple(ntile_w), tuple(tuple(p) for p in pairs_w))
    return per_core, struct


# --------------------------------------------------------------------------
# program builder
# --------------------------------------------------------------------------

def _build(struct, debug=False):
    ntile_w, pairs_w = struct
    TT = sum(ntile_w)
    TP = sum(len(p) for p in pairs_w)
    nc = bacc.Bacc("TRN2", target_bir_lowering=False, debug=False, num_devices=M)
    b16 = dt.bfloat16
    f32 = dt.float32

    # ---- I/O ----
    xT_bf = nc.dram_tensor("xT_bf", [NS, NSL], b16, kind="ExternalInput")
    Wc = nc.dram_tensor("Wc", [NS, 512], b16, kind="ExternalInput")
    Bc = nc.dram_tensor("Bc", [1, 512], b16, kind="ExternalInput")
    S_in = nc.dram_tensor("S_all", [P, TT * P], b16, kind="ExternalInput")
    ST_in = nc.dram_tensor("ST_all", [P, TT * P], b16, kind="ExternalInput")
    E_in = nc.dram_tensor("E_all", [P, TP * P], b16, kind="ExternalInput")
    ea_t = nc.dram_tensor("ea_t", [P, TT], f32, kind="ExternalInput")
    We_row = nc.dram_tensor("We_row", [1, D], f32, kind="ExternalInput")
    xtT_in = nc.dram_tensor("xtT_bf", [D, NTL], b16, kind="ExternalInput")
    q2w_in = nc.dram_tensor("q2w", [D, D], b16, kind="ExternalInput")
    q2b_in = nc.dram_tensor("q2b", [1, D], b16, kind="ExternalInput")
    k2w_in = nc.dram_tensor("k2w", [D, D], b16, kind="ExternalInput")
    k2b_in = nc.dram_tensor("k2b", [1, D], b16, kind="ExternalInput")
    sk2w_in = nc.dram_tensor("sk2w", [D, D], b16, kind="ExternalInput")
    sk2b_in = nc.dram_tensor("sk2b", [1, D], b16, kind="ExternalInput")
    v2wa_in = nc.dram_tensor("v2wa", [D, 132], b16, kind="ExternalInput")
    v2ba_in = nc.dram_tensor("v2ba", [1, 132], b16, kind="ExternalInput")
    gn1_in = nc.dram_tensor("gn1_cols", [D, 3], f32, kind="ExternalInput")
    gn2_in = nc.dram_tensor("gn2_cols", [D, 3], f32, kind="ExternalInput")
    ones_in = nc.dram_tensor("ones_bf", [1, 512], b16, kind="ExternalInput")

    adj_out = nc.dram_tensor("adj_out", [NTL, NT], f32, kind="ExternalOutput")
    if debug:
        dbg_kv = nc.dram_tensor("dbg_kv", [NSL, 256], b16, kind="ExternalOutput")
        dbg_hT = nc.dram_tensor("dbg_hT", [P, NSL], b16, kind="ExternalOutput")
        dbg_xtp = nc.dram_tensor("dbg_xtp", [P, NTL], f32, kind="ExternalOutput")

    # internal DRAM
    kv_loc_w = [nc.dram_tensor(f"kv_loc{w}", [P, 256], b16) for w in range(NWIN)]
    kv_g_w = [nc.dram_tensor(f"kv_g{w}", [M * P, 256], b16, addr_space="Shared")
              for w in range(NWIN)]
    hT_loc_w = [nc.dram_tensor(f"hT_loc{w}", [P, P], b16) for w in range(NWIN)]
    hT_g_w = [nc.dram_tensor(f"hT_g{w}", [M * P, P], b16, addr_space="Shared")
              for w in range(NWIN)]
    st_loc = nc.dram_tensor("st_loc", [P, 8], f32)
    st_full = nc.dram_tensor("st_full", [P, 8], f32, addr_space="Shared")
    xtn_loc = nc.dram_tensor("xtn_loc", [P, NTL], b16)
    xtn_stack = nc.dram_tensor("xtn_stack", [M * P, NTL], b16, addr_space="Shared")
    mm_loc = nc.dram_tensor("mm_loc", [1, 8], f32)
    mm_full = nc.dram_tensor("mm_full", [1, 8], f32, addr_space="Shared")

    rg = [list(range(M))]

    with tile.TileContext(nc) as tc:
        with (
            tc.tile_pool(name="persist", bufs=1) as pp,
        ):
            # ---- persistent small tiles ----
            ea_sb = pp.tile([P, TT], f32, tag="ea")
            nc.sync.dma_start(ea_sb[:], ea_t[:])
            ones_sb = pp.tile([1, 512], b16, tag="ones")
            nc.sync.dma_start(ones_sb[:], ones_in[:])
            We_sb = pp.tile([1, D], f32, tag="We")
            nc.sync.dma_start(We_sb[:], We_row[:])
            gn1_sb = pp.tile([D, 3], f32, tag="gn1")
            nc.sync.dma_start(gn1_sb[:], gn1_in[:])
            gn2_sb = pp.tile([D, 3], f32, tag="gn2")
            nc.sync.dma_start(gn2_sb[:], gn2_in[:])
            ident = pp.tile([P, P], f32, tag="ident")
            make_identity(nc, ident)
            ones_f32_row = pp.tile([1, P], f32, tag="ones_f32")
            nc.vector.memset(ones_f32_row[:], 1.0)
            skip_sb = pp.tile([P, NWIN * D], f32, tag="skip1")
            qq_sb = pp.tile([P, NWIN * 132], b16, tag="qq")
            hT_lsb = pp.tile([P, NSL], b16, tag="hT_lsb")
            We_rep = pp.tile([P, P], f32, tag="We_rep")
            kv_sb = pp.tile([P, NBLK * 256], b16, tag="kv_sb")

            def heat(n, pool):
                for _ in range(n):
                    hps = pool.tile([P, 512], f32, space="PSUM", tag="ps")
                    nc.tensor.matmul(hps[:], ones_sb[:, :P], ones_sb[:],
                                     start=True, stop=True)


            # ================= P1 =================
            with (
                tc.tile_pool(name="p1w", bufs=1) as wp,
                tc.tile_pool(name="p1sb", bufs=3) as p1,
                tc.tile_pool(name="p1ps", bufs=2, space="PSUM") as p1ps,
                tc.tile_pool(name="p1ps2", bufs=2, space="PSUM") as p1ps2,
            ):
                xres = wp.tile([P, 32 * NSL], b16, tag="xres")
                nc.sync.dma_start(
                    xres[:].rearrange("p (k n) -> p k n", k=32),
                    xT_bf[:].rearrange("(k p) n -> p k n", p=P))
                Wc_sb = wp.tile([P, 32 * 512], b16, tag="Wc")
                nc.sync.dma_start(
                    Wc_sb[:].rearrange("p (k n) -> p k n", k=32),
                    Wc[:].rearrange("(k p) n -> p k n", p=P))
                Bc_sb = wp.tile([1, 512], b16, tag="Bc")
                nc.sync.dma_start(Bc_sb[:], Bc[:])

                wr_ps = p1ps.tile([P, 256], f32, space="PSUM", tag="wr")
                nc.tensor.matmul(wr_ps[:, :P], ones_f32_row[:], We_sb[:],
                                 start=True, stop=True)
                nc.vector.tensor_copy(We_rep[:], wr_ps[:, :P])

                for mt in range(NWIN):
                    ps = p1ps2.tile([P, 512], f32, space="PSUM", tag="ps")
                    nc.tensor.matmul(ps[:], ones_sb[:, :P], Bc_sb[:],
                                     start=True, stop=False)
                    for kt in range(32):
                        nc.tensor.matmul(
                            ps[:],
                            xres[:, kt * NSL + mt * P:kt * NSL + (mt + 1) * P],
                            Wc_sb[:, kt * 512:(kt + 1) * 512],
                            start=False, stop=(kt == 31))
                    # evacuate: q | skip | kv ; qWe on DVE
                    nc.vector.tensor_copy(qq_sb[:, mt * 132:mt * 132 + D],
                                          ps[:, 0:D])
                    qwet = p1.tile([P, D], f32, tag="qwet")
                    nc.vector.tensor_tensor(out=qwet[:], in0=ps[:, 0:D],
                                            in1=We_rep[:], op=OP.mult)
                    qwe4 = p1.tile([P, H], f32, tag="qwe4")
                    nc.vector.reduce_sum(
                        out=qwe4[:],
                        in_=qwet[:].rearrange("p (h c) -> p h c", h=H),
                        axis=AX.X)
                    nc.vector.tensor_copy(
                        qq_sb[:, mt * 132 + D:(mt + 1) * 132], qwe4[:])
                    nc.vector.tensor_copy(skip_sb[:, mt * D:(mt + 1) * D],
                                          ps[:, D:2 * D])
                    kv_st = p1.tile([P, 256], b16, tag="kvst")
                    nc.vector.tensor_copy(kv_st[:], ps[:, 2 * D:512])
                    nc.sync.dma_start(kv_loc_w[mt][:], kv_st[:])
                    nc.gpsimd.collective_compute(
                        "AllGather", OP.bypass,
                        ins=[kv_loc_w[mt][:]], outs=[kv_g_w[mt][:]],
                        replica_groups=rg)

                # kv table to SBUF: block b = core b//NWIN, window b%NWIN
                for w in range(NWIN):
                    nc.sync.dma_start(
                        kv_sb[:].rearrange("p (bb n) -> p bb n", n=256)[:, w::NWIN, :],
                        kv_g_w[w][:].rearrange("(r p) n -> p r n", p=P))
                if debug:
                    dkv = p1.tile([P, 256], b16, tag="dkvf")
                    for mt in range(NWIN):
                        nc.sync.dma_start(dkv[:], kv_loc[mt * P:(mt + 1) * P, :])
                        nc.sync.dma_start(dbg_kv[mt * P:(mt + 1) * P, :], dkv[:])

            # ================= edge phase =================
            with (
                tc.tile_pool(name="eoh", bufs=2) as eo,       # one-hot streams
                tc.tile_pool(name="egk", bufs=2) as eg,       # gkv / qx sbuf
                tc.tile_pool(name="escr", bufs=2) as es,
                tc.tile_pool(name="erhs", bufs=2) as er,
                tc.tile_pool(name="egps", bufs=3, space="PSUM") as gps,
                tc.tile_pool(name="eqps", bufs=2, space="PSUM") as qps_p,
                tc.tile_pool(name="eaggps", bufs=2, space="PSUM") as aps,
                tc.tile_pool(name="etrps", bufs=1, space="PSUM") as tps,
            ):
                toff = 0
                poff = 0
                for w in range(NWIN):
                    ntw = npair = None
                    ntw = ntile_w[w]
                    pl = pairs_w[w]
                    npair = len(pl)
                    S_sb = eo.tile([P, ntw * P], b16, tag="S")
                    nc.sync.dma_start(S_sb[:], S_in[:, toff * P:(toff + ntw) * P])
                    ST_sb = eo.tile([P, ntw * P], b16, tag="ST")
                    nc.sync.dma_start(ST_sb[:], ST_in[:, toff * P:(toff + ntw) * P])
                    E_sb = eo.tile([P, npair * P], b16, tag="E")
                    nc.sync.dma_start(E_sb[:], E_in[:, poff * P:(poff + npair) * P])

                    gkvK = eg.tile([P, ntw * P], b16, tag="gkvK")
                    gkvV = eg.tile([P, ntw * P], b16, tag="gkvV")
                    qx = eg.tile([P, ntw * 132], b16, tag="qx")

                    # group pairs by tile
                    by_tile = [[] for _ in range(ntw)]
                    for i, (j, b) in enumerate(pl):
                        by_tile[j].append((i, b))

                    # E-mm + qexp, two tiles per PSUM buffer
                    for j0 in range(0, ntw, 2):
                        jn = min(2, ntw - j0)
                        gkv_ps = gps.tile([P, 512], f32, space="PSUM", tag="g")
                        qe_ps = qps_p.tile([P, 264], f32, space="PSUM", tag="q")
                        for jj in range(jn):
                            j = j0 + jj
                            for idx, (i, b) in enumerate(by_tile[j]):
                                nc.tensor.matmul(
                                    gkv_ps[:, jj * 256:(jj + 1) * 256],
                                    E_sb[:, i * P:(i + 1) * P],
                                    kv_sb[:, b * 256:(b + 1) * 256],
                                    start=(idx == 0),
                                    stop=(idx == len(by_tile[j]) - 1))
                            nc.tensor.matmul(
                                qe_ps[:, jj * 132:(jj + 1) * 132],
                                ST_sb[:, j * P:(j + 1) * P],
                                qq_sb[:, w * 132:(w + 1) * 132],
                                start=True, stop=True)
                        gv3 = gkv_ps[:].rearrange("p (t e) -> p t e", e=256)
                        nc.scalar.activation(
                            gkvK[:, j0 * P:(j0 + jn) * P].rearrange(
                                "p (t e) -> p t e", e=P),
                            gv3[:, 0:jn, 0:P], AF.Copy)
                        nc.vector.tensor_copy(
                            gkvV[:, j0 * P:(j0 + jn) * P].rearrange(
                                "p (t e) -> p t e", e=P),
                            gv3[:, 0:jn, P:256])
                        if (j0 // 2) % 2 == 0:
                            nc.vector.tensor_copy(
                                qx[:, j0 * 132:(j0 + jn) * 132],
                                qe_ps[:, 0:jn * 132])
                        else:
                            nc.scalar.activation(
                                qx[:, j0 * 132:(j0 + jn) * 132],
                                qe_ps[:, 0:jn * 132], AF.Copy)

                    rhs = er.tile([P, ntw * 136], b16, tag="rhs")
                    rhs3 = rhs[:].rearrange("p (t x) -> p t x", x=136)
                    agg_ps = aps.tile([P, 136], f32, space="PSUM", tag="agg")

                    h0 = (ntw + 1) // 2
                    for hb, (j0, nj) in enumerate([(0, h0), (h0, ntw - h0)]):
                        if nj == 0:
                            continue
                        sl = slice(j0, j0 + nj)
                        qx3 = qx[:].rearrange("p (t e) -> p t e", e=132)
                        qk = es.tile([P, nj * D], b16, tag="qk")
                        nc.vector.tensor_tensor(
                            out=qk[:].rearrange("p (t e) -> p t e", e=D),
                            in0=qx3[:, sl, 0:D],
                            in1=gkvK[:, j0 * P:(j0 + nj) * P].rearrange(
                                "p (t e) -> p t e", e=D),
                            op=OP.mult)
                        alpha = es.tile([P, nj * H], f32, tag="al")
                        nc.vector.reduce_sum(
                            out=alpha[:],
                            in_=qk[:].rearrange("p (g c) -> p g c", c=C),
                            axis=AX.X)
                        ea3 = ea_sb[:, toff + j0:toff + j0 + nj]
                        awe = es.tile([P, nj * H], f32, tag="awe")
                        nc.vector.tensor_tensor(
                            out=awe[:].rearrange("p (t h) -> p t h", h=H),
                            in0=qx3[:, sl, D:D + H],
                            in1=ea3.unsqueeze(2).to_broadcast([P, nj, H]),
                            op=OP.mult)
                        nc.vector.tensor_tensor(
                            out=alpha[:], in0=alpha[:], in1=awe[:], op=OP.add)
                        nc.scalar.activation(
                            rhs3[:, sl, 0:H],
                            alpha[:].rearrange("p (t h) -> p t h", h=H),
                            AF.Exp)
                        nc.vector.tensor_tensor(
                            out=rhs3[:, sl, H:2 * H], in0=rhs3[:, sl, 0:H],
                            in1=ea3.unsqueeze(2).to_broadcast([P, nj, H]),
                            op=OP.mult)
                        exx = es.tile([P, nj * D], b16, tag="exx")
                        nc.scalar.activation(
                            exx[:].rearrange("p (t h c) -> p t h c", h=H, c=C),
                            rhs3[:, sl, 0:H].unsqueeze(3)
                                .to_broadcast([P, nj, H, C]),
                            AF.Copy)
                        nc.vector.tensor_tensor(
                            out=rhs3[:, sl, 2 * H:136],
                            in0=gkvV[:, j0 * P:(j0 + nj) * P].rearrange(
                                "p (t e) -> p t e", e=D),
                            in1=exx[:].rearrange("p (t e) -> p t e", e=D),
                            op=OP.mult)

                    for j in range(ntw):
                        nc.tensor.matmul(
                            agg_ps[:],
                            S_sb[:, j * P:(j + 1) * P],
                            rhs[:, j * 136:(j + 1) * 136],
                            start=(j == 0), stop=(j == ntw - 1))

                    # ---- finalize window ----
                    invd = es.tile([P, H], f32, tag="invd")
                    nc.vector.reciprocal(invd[:], agg_ps[:, 0:H])
                    hpre = es.tile([P, D], f32, tag="hpre")
                    nc.vector.tensor_tensor(
                        out=hpre[:].rearrange("p (h c) -> p h c", h=H),
                        in0=agg_ps[:, H:2 * H].unsqueeze(2).to_broadcast([P, H, C]),
                        in1=We_rep[:].rearrange("p (h c) -> p h c", h=H),
                        op=OP.mult)
                    nc.vector.tensor_tensor(
                        out=hpre[:], in0=hpre[:], in1=agg_ps[:, 2 * H:136],
                        op=OP.add)
                    nc.vector.tensor_tensor(
                        out=hpre[:].rearrange("p (h c) -> p h c", h=H),
                        in0=hpre[:].rearrange("p (h c) -> p h c", h=H),
                        in1=invd[:].unsqueeze(2).to_broadcast([P, H, C]),
                        op=OP.mult)
                    nc.vector.tensor_tensor(
                        out=hpre[:], in0=hpre[:],
                        in1=skip_sb[:, w * D:(w + 1) * D], op=OP.add)
                    tr_ps = tps.tile([P, P], f32, space="PSUM", tag="tr")
                    nc.tensor.transpose(tr_ps[:], hpre[:], ident[:])
                    nc.vector.tensor_copy(hT_lsb[:, w * P:(w + 1) * P], tr_ps[:])
                    nc.sync.dma_start(hT_loc_w[w][:], hT_lsb[:, w * P:(w + 1) * P])
                    nc.gpsimd.collective_compute(
                        "AllGather", OP.bypass,
                        ins=[hT_loc_w[w][:]], outs=[hT_g_w[w][:]],
                        replica_groups=rg)
                    toff += ntw
                    poff += npair

            # ================= gn1 =================
            with tc.tile_pool(name="late", bufs=1) as lp:
              xtT_sb = lp.tile([D, NTL], b16, tag="xtT")
              nc.sync.dma_start(xtT_sb[:], xtT_in[:])
              k2T_sb = lp.tile([D, NS], b16, tag="k2T")
              q2T_sb = lp.tile([D, NTL], b16, tag="q2T")
              skT_sb = lp.tile([D, NTL], f32, tag="skT")
              hrelu = lp.tile([D, NS], b16, tag="hrelu")
              hrelu_loc = lp.tile([D, NSL], b16, tag="hrelu_loc")
              with (
                tc.tile_pool(name="gsb", bufs=1) as gs,
                tc.tile_pool(name="gscr", bufs=2) as gsc,
              ):
                hT_pre = gs.tile([P, NS], b16, tag="hT_pre")
                for w in range(NWIN):
                    nc.sync.dma_start(
                        hT_pre[:].rearrange(
                            "p (r w2 n) -> p r w2 n", w2=NWIN, n=P)[:, :, w, :],
                        hT_g_w[w][:].rearrange("(r p) n -> p r n", p=P))
                ssum = gsc.tile([P, 1], f32, tag="ssum")
                scr = gsc.tile([P, NS], b16, tag="scr")
                nc.scalar.activation(scr[:], hT_pre[:], AF.Copy,
                                     accum_out=ssum[:])
                mean = gsc.tile([P, 4], f32, tag="mean")
                nc.vector.tensor_scalar_mul(mean[:, 0:1], ssum[:], float(1.0 / NS))
                nc.vector.tensor_tensor(out=mean[:, 1:2], in0=mean[:, 0:1],
                                        in1=gn1_sb[:, 2:3], op=OP.mult)
                xc = gs.tile([P, NS], b16, tag="xc")
                nc.vector.tensor_scalar_sub(xc[:], hT_pre[:], mean[:, 1:2])
                sumsq = gsc.tile([P, 1], f32, tag="sumsq")
                nc.scalar.activation(scr[:], xc[:], AF.Square, accum_out=sumsq[:])
                var = gsc.tile([P, 1], f32, tag="var")
                nc.vector.tensor_scalar(
                    out=var[:], in0=sumsq[:], scalar1=float(1.0 / NS),
                    scalar2=float(EPS_GN), op0=OP.mult, op1=OP.add)
                nc.scalar.sqrt(var[:], var[:])
                rstd = gsc.tile([P, 1], f32, tag="rstd")
                nc.vector.reciprocal(rstd[:], var[:])
                scale1 = gsc.tile([P, 1], f32, tag="scale1")
                nc.vector.tensor_tensor(out=scale1[:], in0=gn1_sb[:, 0:1],
                                        in1=rstd[:], op=OP.mult)
                nc.scalar.activation(hrelu[:], xc[:], AF.Relu,
                                     bias=gn1_sb[:, 1:2], scale=scale1[:, 0:1])
                xcl = gsc.tile([P, NSL], b16, tag="xcl")
                nc.vector.tensor_scalar_sub(xcl[:], hT_lsb[:], mean[:, 1:2])
                nc.scalar.activation(hrelu_loc[:], xcl[:], AF.Relu,
                                     bias=gn1_sb[:, 1:2], scale=scale1[:, 0:1])
            if debug:
                nc.sync.dma_start(dbg_hT[:], hT_lsb[:])

            # ================= stage 2 =================
            with (
                tc.tile_pool(name="s2w", bufs=1) as s2w,
                tc.tile_pool(name="s2ps", bufs=2, space="PSUM") as s2ps,
                tc.tile_pool(name="s2sc", bufs=2) as s2c,
            ):
                k2w_sb = s2w.tile([D, D], b16, tag="k2w")
                nc.sync.dma_start(k2w_sb[:], k2w_in[:])
                k2b_sb = s2w.tile([1, D], b16, tag="k2b")
                nc.sync.dma_start(k2b_sb[:], k2b_in[:])
                q2w_sb = s2w.tile([D, D], b16, tag="q2w")
                nc.sync.dma_start(q2w_sb[:], q2w_in[:])
                q2b_sb = s2w.tile([1, D], b16, tag="q2b")
                nc.sync.dma_start(q2b_sb[:], q2b_in[:])
                sk2w_sb = s2w.tile([D, D], b16, tag="sk2w")
                nc.sync.dma_start(sk2w_sb[:], sk2w_in[:])
                sk2b_sb = s2w.tile([1, D], b16, tag="sk2b")
                nc.sync.dma_start(sk2b_sb[:], sk2b_in[:])
                v2wa_sb = s2w.tile([D, 132], b16, tag="v2wa")
                nc.sync.dma_start(v2wa_sb[:], v2wa_in[:])
                v2ba_sb = s2w.tile([1, 132], b16, tag="v2ba")
                nc.sync.dma_start(v2ba_sb[:], v2ba_in[:])

                qps = s2ps.tile([P, 512], f32, space="PSUM", tag="ps")
                nc.tensor.matmul(qps[:, :NTL], q2b_sb[:], ones_sb[:, :NTL],
                                 start=True, stop=False)
                nc.tensor.matmul(qps[:, :NTL], q2w_sb[:], xtT_sb[:],
                                 start=False, stop=True)
                nc.vector.tensor_copy(q2T_sb[:], qps[:, :NTL])
                q2z_sb = s2w.tile([D, H * NTL], b16, tag="q2z")
                nc.vector.memset(q2z_sb[:], 0.0)
                for h in range(H):
                    nc.vector.tensor_copy(
                        q2z_sb[32 * h:32 * (h + 1), h * NTL:(h + 1) * NTL],
                        q2T_sb[32 * h:32 * (h + 1), :])
                sps = s2ps.tile([P, 512], f32, space="PSUM", tag="ps")
                nc.tensor.matmul(sps[:, :NTL], sk2b_sb[:], ones_sb[:, :NTL],
                                 start=True, stop=False)
                nc.tensor.matmul(sps[:, :NTL], sk2w_sb[:], xtT_sb[:],
                                 start=False, stop=True)
                nc.vector.tensor_copy(skT_sb[:], sps[:, :NTL])

                for ch in range(8):
                    kps = s2ps.tile([P, 512], f32, space="PSUM", tag="ps")
                    nc.tensor.matmul(kps[:], k2b_sb[:], ones_sb[:],
                                     start=True, stop=False)
                    nc.tensor.matmul(kps[:], k2w_sb[:],
                                     hrelu[:, ch * 512:(ch + 1) * 512],
                                     start=False, stop=True)
                    if ch % 2 == 0:
                        nc.scalar.activation(
                            k2T_sb[:, ch * 512:(ch + 1) * 512], kps[:], AF.Copy)
                    else:
                        nc.vector.tensor_copy(
                            k2T_sb[:, ch * 512:(ch + 1) * 512], kps[:])

                st_sb = s2c.tile([P, 8], f32, tag="st")
                nc.vector.memset(st_sb[:], 0.0)
                scr2 = s2c.tile([P, 512], b16, tag="scr2")
                gps2 = s2ps.tile([P, 512], f32, space="PSUM", tag="ps")
                nc.tensor.matmul(gps2[:], sk2b_sb[:], ones_sb[:],
                                 start=True, stop=False)
                nc.tensor.matmul(gps2[:], sk2w_sb[:], hrelu_loc[:],
                                 start=False, stop=True)
                nc.vector.reduce_sum(out=st_sb[:, 0:1], in_=gps2[:], axis=AX.X)
                nc.scalar.activation(scr2[:], gps2[:], AF.Square,
                                     accum_out=st_sb[:, 1:2])

                v2a_all = s2w.tile([P, 32 * 132], b16, tag="v2a_all")
                for st in range(32):
                    vps = s2ps.tile([P, 512], f32, space="PSUM", tag="ps")
                    nc.tensor.matmul(vps[:, :132], ones_sb[:, :P], v2ba_sb[:],
                                     start=True, stop=False)
                    nc.tensor.matmul(vps[:, :132],
                                     hrelu[:, st * P:(st + 1) * P],
                                     v2wa_sb[:], start=False, stop=True)
                    if st % 2 == 0:
                        nc.scalar.activation(
                            v2a_all[:, st * 132:(st + 1) * 132], vps[:, :132],
                            AF.Copy)
                    else:
                        nc.vector.tensor_copy(
                            v2a_all[:, st * 132:(st + 1) * 132], vps[:, :132])

                # ---- attention s-loop ----
                with (
                    tc.tile_pool(name="scps", bufs=2, space="PSUM") as scp,
                    tc.tile_pool(name="ndps", bufs=1, space="PSUM") as ndp,
                    tc.tile_pool(name="sexp", bufs=3) as sxp,
                ):
                    nd_ps = ndp.tile([33, H * NTL], f32, space="PSUM", tag="nd")
                    for st in range(32):
                        scps = scp.tile([P, H * NTL], f32, space="PSUM", tag="sc")
                        for half in range(2):
                            nc.tensor.matmul(
                                scps[:, half * 512:(half + 1) * 512],
                                k2T_sb[:, st * P:(st + 1) * P],
                                q2z_sb[:, half * 512:(half + 1) * 512],
                                start=True, stop=True)
                        exs = sxp.tile([P, H * NTL], b16, tag="exs")
                        nc.scalar.activation(exs[:], scps[:], AF.Exp)
                        for h in range(H):
                            nc.tensor.matmul(
                                nd_ps[:, h * NTL:(h + 1) * NTL],
                                v2a_all[:, st * 132 + 33 * h:st * 132 + 33 * (h + 1)],
                                exs[:, h * NTL:(h + 1) * NTL],
                                start=(st == 0), stop=(st == 31))

                    xtpT = s2c.tile([D, NTL], f32, tag="xtpT")
                    for h in range(H):
                        denrow = s2c.tile([1, NTL], f32, tag="denrow")
                        nc.vector.tensor_copy(denrow[:],
                                              nd_ps[32:33, h * NTL:(h + 1) * NTL])
                        drep = s2ps.tile([P, 512], f32, space="PSUM", tag="ps")
                        nc.tensor.matmul(drep[:, :NTL], ones_f32_row[:],
                                         denrow[:], start=True, stop=True)
                        invd2 = s2c.tile([32, NTL], f32, tag="invd2")
                        nc.vector.reciprocal(invd2[:], drep[:32, :NTL])
                        nc.vector.tensor_tensor(
                            out=xtpT[32 * h:32 * (h + 1), :],
                            in0=nd_ps[0:32, h * NTL:(h + 1) * NTL],
                            in1=invd2[:], op=OP.mult)
                    nc.vector.tensor_tensor(out=xtpT[:], in0=xtpT[:],
                                            in1=skT_sb[:], op=OP.add)
                if debug:
                    nc.sync.dma_start(dbg_xtp[:], xtpT[:])

                tsum = s2c.tile([P, 2], f32, tag="tsum")
                nc.vector.reduce_sum(out=tsum[:, 0:1], in_=xtpT[:], axis=AX.X)
                scr3 = s2c.tile([P, NTL], b16, tag="scr3")
                nc.scalar.activation(scr3[:], xtpT[:], AF.Square,
                                     accum_out=tsum[:, 1:2])
                nc.vector.tensor_tensor(out=st_sb[:, 0:2], in0=st_sb[:, 0:2],
                                        in1=tsum[:], op=OP.add)
                nc.sync.dma_start(st_loc[:], st_sb[:])
                nc.gpsimd.collective_compute(
                    "AllReduce", OP.add,
                    ins=[st_loc[:]], outs=[st_full[:]], replica_groups=rg)
                stf = s2c.tile([P, 8], f32, tag="stf")
                nc.sync.dma_start(stf[:], st_full[:])

                NALL = float(NS + NT)
                t2 = s2c.tile([P, 6], f32, tag="t2")
                mean2 = t2[:, 0:1]
                nc.vector.tensor_scalar_mul(mean2, stf[:, 0:1], float(1.0 / NALL))
                nc.vector.tensor_scalar(
                    out=t2[:, 1:2], in0=gn2_sb[:, 2:3], scalar1=-1.0, scalar2=2.0,
                    op0=OP.mult, op1=OP.add)
                nc.vector.tensor_tensor(out=t2[:, 1:2], in0=t2[:, 1:2],
                                        in1=gn2_sb[:, 2:3], op=OP.mult)
                nc.vector.tensor_tensor(out=t2[:, 2:3], in0=mean2, in1=mean2,
                                        op=OP.mult)
                nc.vector.tensor_tensor(out=t2[:, 2:3], in0=t2[:, 2:3],
                                        in1=t2[:, 1:2], op=OP.mult)
                var2 = t2[:, 3:4]
                nc.vector.tensor_scalar_mul(var2, stf[:, 1:2], float(1.0 / NALL))
                nc.vector.tensor_tensor(out=var2, in0=var2, in1=t2[:, 2:3],
                                        op=OP.subtract)
                nc.vector.tensor_scalar_add(var2, var2, float(EPS_GN))
                nc.scalar.sqrt(var2, var2)
                rstd2 = t2[:, 4:5]
                nc.vector.reciprocal(rstd2, var2)
                scale2 = s2c.tile([P, 2], f32, tag="sc2")
                nc.vector.tensor_tensor(out=scale2[:, 0:1], in0=gn2_sb[:, 0:1],
                                        in1=rstd2, op=OP.mult)
                nc.vector.tensor_tensor(out=t2[:, 5:6], in0=mean2,
                                        in1=gn2_sb[:, 2:3], op=OP.mult)
                nc.vector.tensor_tensor(out=t2[:, 5:6], in0=t2[:, 5:6],
                                        in1=scale2[:, 0:1], op=OP.mult)
                nc.vector.tensor_scalar_mul(t2[:, 5:6], t2[:, 5:6], -1.0)
                nc.vector.tensor_tensor(out=scale2[:, 1:2], in0=gn2_sb[:, 1:2],
                                        in1=t2[:, 5:6], op=OP.add)

                xtnT = s2c.tile([D, NTL], b16, tag="xtnT")
                nc.scalar.activation(xtnT[:], xtpT[:], AF.Relu,
                                     bias=scale2[:, 1:2], scale=scale2[:, 0:1])
                nc.sync.dma_start(xtn_loc[:], xtnT[:])
                nc.gpsimd.collective_compute(
                    "AllGather", OP.bypass,
                    ins=[xtn_loc[:]], outs=[xtn_stack[:]], replica_groups=rg)

                xtfT = s2w.tile([D, NT], b16, tag="xtfT")
                for r in range(M):
                    nc.sync.dma_start(xtfT[:, r * NTL:(r + 1) * NTL],
                                      xtn_stack[r * P:(r + 1) * P, :])
                adj_sb = s2w.tile([P, 2 * NT], f32, tag="adj")
                mxc = s2c.tile([P, 2], f32, tag="mxc")
                first = True
                for mt in range(2):
                    for nk in range(4):
                        adps = s2ps.tile([P, 512], f32, space="PSUM", tag="ps")
                        nc.tensor.matmul(
                            adps[:],
                            xtnT[:, mt * P:(mt + 1) * P],
                            xtfT[:, nk * 512:(nk + 1) * 512],
                            start=True, stop=True)
                        nc.vector.tensor_copy(
                            adj_sb[:, mt * NT + nk * 512:mt * NT + (nk + 1) * 512],
                            adps[:])
                        tmx = s2c.tile([P, 2], f32, tag="tmx")
                        nc.vector.reduce_max(out=tmx[:, 0:1], in_=adps[:],
                                             axis=AX.X)
                        nc.vector.tensor_reduce(
                            out=tmx[:, 1:2], in_=adps[:], op=OP.min, axis=AX.X)
                        if first:
                            nc.vector.tensor_copy(mxc[:], tmx[:])
                            first = False
                        else:
                            nc.vector.tensor_tensor(
                                out=mxc[:, 0:1], in0=mxc[:, 0:1],
                                in1=tmx[:, 0:1], op=OP.max)
                            nc.vector.tensor_tensor(
                                out=mxc[:, 1:2], in0=mxc[:, 1:2],
                                in1=tmx[:, 1:2], op=OP.min)
                nc.vector.tensor_scalar_mul(mxc[:, 1:2], mxc[:, 1:2], -1.0)
                mxt_ps = s2ps.tile([P, 512], f32, space="PSUM", tag="ps")
                nc.tensor.transpose(mxt_ps[:2, :P], mxc[:], ident[:])
                mxrow = s2c.tile([2, P], f32, tag="mxrow")
                nc.vector.tensor_copy(mxrow[:], mxt_ps[:2, :P])
                mm2 = s2c.tile([2, 1], f32, tag="mm2")
                nc.vector.reduce_max(out=mm2[:], in_=mxrow[:], axis=AX.X)
                mm2t = s2ps.tile([P, 512], f32, space="PSUM", tag="ps")
                nc.tensor.transpose(mm2t[:1, :2], mm2[:], ident[:2, :2])
                mmrow = s2c.tile([1, 8], f32, tag="mmrow")
                nc.vector.memset(mmrow[:], -1e30)
                nc.vector.tensor_copy(mmrow[:, 0:2], mm2t[:1, :2])
                nc.sync.dma_start(mm_loc[:], mmrow[:])
                nc.gpsimd.collective_compute(
                    "AllReduce", OP.max,
                    ins=[mm_loc[:]], outs=[mm_full[:]], replica_groups=rg)
                mmf = s2c.tile([1, 8], f32, tag="mmf")
                nc.sync.dma_start(mmf[:], mm_full[:])
                sc = s2c.tile([1, 4], f32, tag="scl")
                nc.vector.tensor_tensor(out=sc[:, 0:1], in0=mmf[:, 0:1],
                                        in1=mmf[:, 1:2], op=OP.add)
                nc.vector.tensor_scalar_add(sc[:, 0:1], sc[:, 0:1], 1e-8)
                nc.vector.reciprocal(sc[:, 1:2], sc[:, 0:1])
                nc.vector.tensor_scalar_mul(sc[:, 2:3], mmf[:, 1:2], -1.0)
                mnrep = s2ps.tile([P, 512], f32, space="PSUM", tag="ps")
                nc.tensor.matmul(mnrep[:, :2], ones_f32_row[:], sc[:, 1:3],
                                 start=True, stop=True)
                mncol = s2c.tile([P, 2], f32, tag="mncol")
                nc.vector.tensor_copy(mncol[:], mnrep[:, :2])
                for mt in range(2):
                    onorm = s2w.tile([P, NT], f32, tag="onorm")
                    nc.vector.tensor_scalar(
                        out=onorm[:], in0=adj_sb[:, mt * NT:(mt + 1) * NT],
                        scalar1=mncol[:, 1:2], scalar2=mncol[:, 0:1],
                        op0=OP.subtract, op1=OP.mult)
                    nc.sync.dma_start(adj_out[mt * P:(mt + 1) * P, :], onorm[:])

    nc.compile()
    return nc


def _get_prog(struct, debug=False):
    key = (struct, debug)
    if key not in _prog_cache:
        _prog_cache[key] = _build(struct, debug)
    return _prog_cache[key]


def kernel(**inputs):
    per_core, struct = _prep(inputs)
    debug = os.environ.get("KB_DEBUG", "0") == "1"
    nc = _get_prog(struct, debug)
    trace = os.environ.get("KB_TRACE", "0") == "1"
    res = run_bass_kernel_spmd(nc, per_core, core_ids=list(range(M)), trace=trace)
    if trace:
        kernel.last_result = res
    out = np.concatenate([res.results[c]["adj_out"] for c in range(M)], axis=0)
    if debug:
        kernel.debug_results = res.results
    return out
